# revision 1
# baseline (speedup 1.0000x reference)
"""Trainium2 Bass kernel for nn_ConstraintModel (2-LSTM chain + MLP head).

Contract: kernel(**inputs) takes FULL unsharded inputs (numpy, keyed as in
setup_inputs()) and returns the FULL (512, 256, 128) float32 output.

Strategy: data-parallel over batch (256 -> 8 cores x 32). Each core runs an
identical Bass program on its batch shard:
  phase C: constraint LSTM scanned backward over the 512 steps
  phase G: generation LSTM scanned forward, consuming the stored constraint
           hiddens; per-segment MLP head; DMA out.

Layout: everything on chip is kept transposed -- [feature/hidden on SBUF
partitions, batch on the free dim] -- so the recurrent matmuls produce
gates.T directly, elementwise gate math runs on all 128 partitions, and the
new hidden state feeds the next step's matmul with no transposes anywhere.
The host pre-transposes / gate-permutes all inputs and weights.

The scan is software-pipelined by hidden half: gate blocks are ordered
(i,f,o,g | half0, then half1), the recurrent matmuls are emitted k-outer so
all half0-consuming matmuls of step t+1 only depend on half0 of h_t, and the
elementwise chain computes half0 fully before half1.  h is stored bf16 (the
recurrent matmuls and the gen-LSTM/MLP bulk matmuls consume it directly);
the cell state c stays fp32.
"""

import os
import sys
from contextlib import ExitStack

sys.path.insert(0, "/opt/pypackages")
sys.path.insert(0, "/opt/trn_rl_repo")

import numpy as np
from ml_dtypes import bfloat16

import concourse.bass as bass
import concourse.bacc as bacc
import concourse.tile as tile
from concourse import mybir
from concourse.bass_utils import run_bass_kernel_spmd

F32 = mybir.dt.float32
BF16 = mybir.dt.bfloat16
AF = mybir.ActivationFunctionType
ALU = mybir.AluOpType

S_FULL = 512
B_FULL = 256
F = 128          # seq features
FC = 129         # constraint features
H = 256          # hidden (both LSTMs)
NQ = 8           # 4*H / 128 gate m-tiles
NCORES = 8
BL = B_FULL // NCORES  # 32 batch per core
TSEG = 16        # scan steps per bulk segment

# gate permutation: torch order (i, f, g, o) rows ->
# on-chip blocks (i,f,o,g for hidden half0 | i,f,o,g for half1), 128 rows each
_i, _f, _g, _o = np.r_[0:256], np.r_[256:512], np.r_[512:768], np.r_[768:1024]
GATE_PERM = np.concatenate([
    _i[:128], _f[:128], _o[:128], _g[:128],
    _i[128:], _f[128:], _o[128:], _g[128:],
])


# --------------------------------------------------------------------------
# host-side preparation
# --------------------------------------------------------------------------

def prep_weights(inp: dict) -> dict:
    """Gate-permute + transpose all weights. Shared across cores."""
    g = lambda w: np.ascontiguousarray(np.asarray(w, np.float32)[GATE_PERM])
    out = {}
    out["wihc"] = np.ascontiguousarray(g(inp["Wih_c"]).T)            # [129,1024]
    out["whhc"] = np.ascontiguousarray(g(inp["Whh_c"]).T).astype(bfloat16)
    wg = g(inp["Wih_g"])                                             # [1024, 384]
    out["wgx"] = np.ascontiguousarray(wg[:, :F].T)                   # [128, 1024]
    out["wghc"] = np.ascontiguousarray(wg[:, F:].T).astype(bfloat16) # [256, 1024]
    out["whhg"] = np.ascontiguousarray(g(inp["Whh_g"]).T).astype(bfloat16)
    out["w1t"] = np.ascontiguousarray(
        np.asarray(inp["W1"], np.float32).T).astype(bfloat16)        # [256, 128]
    out["w2t"] = np.ascontiguousarray(np.asarray(inp["W2"], np.float32).T)
    bc = np.asarray(inp["bih_c"], np.float32) + np.asarray(inp["bhh_c"], np.float32)
    bg = np.asarray(inp["bih_g"], np.float32) + np.asarray(inp["bhh_g"], np.float32)
    out["bc"] = np.ascontiguousarray(bc[GATE_PERM].reshape(NQ, 128).T)  # [128, 8]
    out["bg"] = np.ascontiguousarray(bg[GATE_PERM].reshape(NQ, 128).T)  # [128, 8]
    out["b1"] = np.ascontiguousarray(np.asarray(inp["b1"], np.float32)[:, None])
    out["b2"] = np.ascontiguousarray(np.asarray(inp["b2"], np.float32)[:, None])
    return out


def prep_core_inputs(seq, seq_constraints, c0, c1, s):
    """Per-core activation tensors for batch columns [c0:c1), seq len s."""
    xc = np.asarray(seq_constraints, np.float32)[:s, c0:c1]   # [s, bl, 129]
    # time-reversed + transposed: xcT[k, tau, b] = xc[s-1-tau, b, k]
    xcT = np.ascontiguousarray(xc[::-1].transpose(2, 0, 1))   # [129, s, bl]
    sq = np.asarray(seq, np.float32)[:s, c0:c1]               # [s, bl, 128]
    shifted = np.concatenate([np.zeros_like(sq[:1]), sq[:-1]], axis=0)
    xgT = np.ascontiguousarray(shifted.transpose(2, 0, 1))    # [128, s, bl]
    return {"xcT": xcT, "xgT": xgT}


# --------------------------------------------------------------------------
# device program
# --------------------------------------------------------------------------

def build_program(s=S_FULL, tseg=TSEG, bl=BL):
    """Build + compile the per-core Bass program. Returns (nc, out_name)."""
    assert s % tseg == 0
    nseg = s // tseg
    nc = bacc.Bacc("TRN2", target_bir_lowering=False, debug=False,
                   enable_asserts=False)

    d_xcT = nc.dram_tensor("xcT", [FC, s, bl], F32, kind="ExternalInput")
    d_xgT = nc.dram_tensor("xgT", [F, s, bl], F32, kind="ExternalInput")
    d_wihc = nc.dram_tensor("wihc", [FC, 4 * H], F32, kind="ExternalInput")
    d_whhc = nc.dram_tensor("whhc", [H, 4 * H], BF16, kind="ExternalInput")
    d_wgx = nc.dram_tensor("wgx", [F, 4 * H], F32, kind="ExternalInput")
    d_wghc = nc.dram_tensor("wghc", [H, 4 * H], BF16, kind="ExternalInput")
    d_whhg = nc.dram_tensor("whhg", [H, 4 * H], BF16, kind="ExternalInput")
    d_w1t = nc.dram_tensor("w1t", [H, F], BF16, kind="ExternalInput")
    d_w2t = nc.dram_tensor("w2t", [F, F], F32, kind="ExternalInput")
    d_bc = nc.dram_tensor("bc", [128, NQ], F32, kind="ExternalInput")
    d_bg = nc.dram_tensor("bg", [128, NQ], F32, kind="ExternalInput")
    d_b1 = nc.dram_tensor("b1", [128, 1], F32, kind="ExternalInput")
    d_b2 = nc.dram_tensor("b2", [128, 1], F32, kind="ExternalInput")
    d_out = nc.dram_tensor("out", [F, s, bl], F32, kind="ExternalOutput")

    with tile.TileContext(nc) as tc, ExitStack() as ctx:
        wp = ctx.enter_context(tc.tile_pool(name="weights", bufs=1))
        hcp = ctx.enter_context(tc.tile_pool(name="hcstore", bufs=1))
        xpp = ctx.enter_context(tc.tile_pool(name="xproj", bufs=2))
        xinp = ctx.enter_context(tc.tile_pool(name="xin", bufs=3))
        hgp = ctx.enter_context(tc.tile_pool(name="hgseg", bufs=2))
        yp = ctx.enter_context(tc.tile_pool(name="yout", bufs=2))
        stp = ctx.enter_context(tc.tile_pool(name="state", bufs=3))
        ewp = ctx.enter_context(tc.tile_pool(name="eltwise", bufs=3))
        psb = ctx.enter_context(tc.tile_pool(name="psbulk", bufs=3,
                                             space=bass.MemorySpace.PSUM))
        psg = ctx.enter_context(tc.tile_pool(name="psgates", bufs=3,
                                             space=bass.MemorySpace.PSUM))

        # ---- load weights to SBUF (resident all kernel) ----
        def wtile(dram, shape, dt=F32, row0=0):
            t = wp.tile(shape, dt, tag=f"w_{dram.name}_{row0}")
            nc.sync.dma_start(t[:], dram.ap()[row0:row0 + shape[0]])
            return t

        wihc_k0 = wtile(d_wihc, [128, 4 * H])
        wihc_k1 = wtile(d_wihc, [1, 4 * H], row0=128)
        whhc = [wtile(d_whhc, [128, 4 * H], BF16, row0=128 * k)
                for k in range(2)]
        wgx = wtile(d_wgx, [128, 4 * H])
        wghc = [wtile(d_wghc, [128, 4 * H], BF16, row0=128 * k) for k in range(2)]
        whhg = [wtile(d_whhg, [128, 4 * H], BF16, row0=128 * k) for k in range(2)]
        w1t = [wtile(d_w1t, [128, F], BF16, row0=128 * k) for k in range(2)]
        w2t = wtile(d_w2t, [128, F])
        bc_sb = wtile(d_bc, [128, NQ])
        bg_sb = wtile(d_bg, [128, NQ])
        b1_sb = wtile(d_b1, [128, 1])
        b2_sb = wtile(d_b2, [128, 1])

        # constraint hiddens for every forward-time index t, bf16, split by
        # hidden half so the pipeline deps stay per-half
        hc = [hcp.tile([128, s, bl], BF16, tag=f"hc{k}", name=f"hc{k}")
              for k in range(2)]

        def scan_step(xp_tile, tl, whh, h_prev, c_prev, h_out):
            """One LSTM step, half-split pipelined.

            h_prev/h_out: [ap_half0, ap_half1] (bf16), c_prev: [t0, t1] fp32
            tiles.  Returns new [c0, c1].
            """
            pg = psg.tile([128, NQ, bl], F32, tag="pg")
            for k in range(2):
                for q in range(NQ):
                    for r in range(4):
                        col = 128 * q + 32 * r
                        # pending-zero is bank-wide per partition strip: the
                        # first matmul of each strip marks the whole bank,
                        # every later k0 write lands on pending-zero bytes
                        # (overwrite-as-zeroed), k1 writes accumulate.
                        nc.tensor.matmul(
                            pg[32 * r:32 * r + 32, q, :],
                            whh[k][:, col:col + 32],
                            h_prev[k],
                            start=(k == 0 and q == 0), stop=(k == 1),
                            tile_position=(0, 32 * r),
                            skip_group_check=True,
                        )
            c_new = []
            for hh in range(2):  # hidden half
                q0 = 4 * hh
                gs = ewp.tile([128, 4, bl], F32, tag=f"gs{hh}")
                nc.vector.tensor_tensor(gs[:], pg[:, q0:q0 + 4],
                                        xp_tile[:, tl, q0:q0 + 4], ALU.add)
                sig = ewp.tile([128, 3, bl], F32, tag=f"sig{hh}")
                nc.scalar.activation(sig[:], gs[:, 0:3], AF.Sigmoid)
                tg = ewp.tile([128, bl], F32, tag=f"tg{hh}")
                nc.scalar.activation(tg[:], gs[:, 3], AF.Tanh)
                u = ewp.tile([128, bl], F32, tag=f"u{hh}")
                nc.vector.tensor_tensor(u[:], sig[:, 0], tg[:], ALU.mult)
                v = ewp.tile([128, bl], F32, tag=f"v{hh}")
                nc.vector.tensor_tensor(v[:], sig[:, 1], c_prev[hh][:],
                                        ALU.mult)
                cn = stp.tile([128, bl], F32, tag=f"c{hh}")
                nc.vector.tensor_tensor(cn[:], u[:], v[:], ALU.add)
                tc2 = ewp.tile([128, bl], F32, tag=f"tc2{hh}")
                nc.scalar.activation(tc2[:], cn[:], AF.Tanh)
                nc.vector.tensor_tensor(h_out[hh], sig[:, 2], tc2[:],
                                        ALU.mult)
                c_new.append(cn)
            return c_new

        def zero_state():
            hz = stp.tile([128, bl], BF16, tag="hz")
            nc.vector.memset(hz[:], 0.0)
            cs = []
            for hh in range(2):
                cz = stp.tile([128, bl], F32, tag=f"c{hh}")
                nc.vector.memset(cz[:], 0.0)
                cs.append(cz)
            return hz, cs

        # =================== phase C: constraint LSTM (backward) ==========
        hz, c_prev = zero_state()
        h_prev = [hz[:], hz[:]]
        for seg in range(nseg):
            t0 = seg * tseg
            xc0 = xinp.tile([128, tseg, bl], F32, tag="xc0")
            nc.sync.dma_start(xc0[:], d_xcT.ap()[0:128, t0:t0 + tseg])
            xc1 = xinp.tile([1, tseg, bl], F32, tag="xc1")
            nc.sync.dma_start(xc1[:], d_xcT.ap()[128:129, t0:t0 + tseg])
            xp = xpp.tile([128, tseg, NQ, bl], F32, tag="xp")
            for q in range(NQ):
                ps = psb.tile([128, tseg, bl], F32, tag="psb")
                nc.tensor.matmul(ps[:], wihc_k0[:, 128 * q:128 * (q + 1)],
                                 xc0[:], start=True, stop=False)
                nc.tensor.matmul(ps[:], wihc_k1[:, 128 * q:128 * (q + 1)],
                                 xc1[:], start=False, stop=True)
                nc.scalar.activation(xp[:, :, q, :], ps[:], AF.Identity,
                                     bias=bc_sb[:, q:q + 1])
            for tl in range(tseg):
                t = s - 1 - (t0 + tl)           # forward-time index
                h_out = [hc[0][:, t], hc[1][:, t]]
                c_prev = scan_step(xp, tl, whhc, h_prev, c_prev, h_out)
                h_prev = h_out

        # =================== phase G: generation LSTM (forward) ===========
        hz, c_prev = zero_state()
        h_prev = [hz[:], hz[:]]
        for seg in range(nseg):
            t0 = seg * tseg
            xg = xinp.tile([128, tseg, bl], F32, tag="xc0")
            nc.sync.dma_start(xg[:], d_xgT.ap()[0:128, t0:t0 + tseg])
            xp = xpp.tile([128, tseg, NQ, bl], F32, tag="xp")
            for q in range(NQ):
                ps = psb.tile([128, tseg, bl], F32, tag="psb")
                nc.tensor.matmul(ps[:], wgx[:, 128 * q:128 * (q + 1)],
                                 xg[:], start=True, stop=False)
                for k in range(2):
                    nc.tensor.matmul(ps[:], wghc[k][:, 128 * q:128 * (q + 1)],
                                     hc[k][:, t0:t0 + tseg],
                                     start=False, stop=(k == 1))
                nc.scalar.activation(xp[:, :, q, :], ps[:], AF.Identity,
                                     bias=bg_sb[:, q:q + 1])
            hg = [hgp.tile([128, tseg, bl], BF16, tag=f"hg{k}",
                           name=f"hg{k}") for k in range(2)]
            for tl in range(tseg):
                h_out = [hg[0][:, tl], hg[1][:, tl]]
                c_prev = scan_step(xp, tl, whhg, h_prev, c_prev, h_out)
                h_prev = h_out
            # ---- MLP head for this segment ----
            ps1 = psb.tile([128, tseg, bl], F32, tag="psb")
            for k in range(2):
                nc.tensor.matmul(ps1[:], w1t[k][:], hg[k][:],
                                 start=(k == 0), stop=(k == 1))
            y1 = yp.tile([128, tseg, bl], F32, tag="y1")
            nc.scalar.activation(y1[:], ps1[:], AF.Relu, bias=b1_sb[:, 0:1])
            ps2 = psb.tile([128, tseg, bl], F32, tag="psb")
            nc.tensor.matmul(ps2[:], w2t[:], y1[:], start=True, stop=True)
            y2 = yp.tile([128, tseg, bl], F32, tag="y2")
            nc.scalar.activation(y2[:], ps2[:], AF.Identity, bias=b2_sb[:, 0:1])
            nc.sync.dma_start(d_out.ap()[:, t0:t0 + tseg], y2[:])

    nc.compile()
    return nc, "out"


_PROGRAM_CACHE = {}


def get_program(s=S_FULL, tseg=TSEG, bl=BL):
    key = (s, tseg, bl)
    if key not in _PROGRAM_CACHE:
        _PROGRAM_CACHE[key] = build_program(s, tseg, bl)
    return _PROGRAM_CACHE[key]


# --------------------------------------------------------------------------
# entry point
# --------------------------------------------------------------------------

def kernel(**inputs) -> np.ndarray:
    s, b = np.asarray(inputs["seq"]).shape[:2]
    assert (s, b) == (S_FULL, B_FULL)
    nc, out_name = get_program()
    w = prep_weights(inputs)
    in_maps = []
    for core in range(NCORES):
        c0 = core * BL
        m = dict(w)
        m.update(prep_core_inputs(inputs["seq"], inputs["seq_constraints"],
                                  c0, c0 + BL, S_FULL))
        in_maps.append(m)
    res = run_bass_kernel_spmd(nc, in_maps, core_ids=list(range(NCORES)))
    # per-core out: [F, S, BL] -> [S, BL, F]; concat cores along batch
    parts = [np.transpose(res.results[c][out_name], (1, 2, 0))
             for c in range(NCORES)]
    return np.ascontiguousarray(np.concatenate(parts, axis=1))



# revision 4
# speedup vs baseline: 1.5980x; 1.5980x over previous
"""Trainium2 Bass kernel for nn_ConstraintModel (2-LSTM chain + MLP head).

Contract: kernel(**inputs) takes FULL unsharded inputs (numpy, keyed as in
setup_inputs()) and returns the FULL (512, 256, 128) float32 output.

Strategy: data-parallel over batch (256 -> 8 cores x 32). Each core runs an
identical Bass program on its batch shard:
  phase C: constraint LSTM scanned backward over the 512 steps
  phase G: generation LSTM scanned forward, consuming the stored constraint
           hiddens
  phase M: bulk MLP head over all stored generation hiddens.

Layout: [feature/hidden on SBUF partitions, batch on the free dim] so the
recurrent matmuls produce gates.T directly and elementwise gate math runs on
all 128 partitions.

Key structure (v2):
- The per-segment input projections (x @ Wih + bias) are accumulated DIRECTLY
  into the per-step gate PSUM regions; the per-step recurrent matmuls then
  accumulate on top (start=False).  No per-step gate add, no PSUM->SBUF
  copies.  Biases ride along as an extra ones-row of the input.
- Recurrent matmul: 16 [128c x 128p x 32f] matmuls per step (2 contraction
  halves x 8 gate tiles), bf16 weights (FWL-eligible 128-col stationaries).
- Gate nonlinearity: ONE tanh activation per hidden half covering all 4 gate
  tiles, using sigmoid(x) = (tanh(x/2)+1)/2.  The 1/2 pre-scale is folded
  into the i/f/o weight rows; the (t+1)/2 fix-ups are folded into fused
  scalar_tensor_tensor ops:
      v2 = (tf + 1) * cs_prev          # = 4*sigm(f)*c_prev   (cs = 2c)
      u2 = (ti + 1) * tg               # = 2*sigm(i)*tanh(g)
      cs = v2*0.5 + u2                 # = 2*c_new
      tc = tanh(0.5 * cs)              # = tanh(c_new)
      H2 = (to + 1) * tc               # = 2*h_new
  h is stored scaled by 2 (bf16); every weight column that consumes h is
  pre-scaled by 0.5 on the host, making the convention exact.
"""

import sys
from contextlib import ExitStack

sys.path.insert(0, "/opt/pypackages")
sys.path.insert(0, "/opt/trn_rl_repo")

import numpy as np
from ml_dtypes import bfloat16

import concourse.bass as bass
import concourse.bacc as bacc
import concourse.tile as tile
from concourse import mybir
from concourse.bass_utils import run_bass_kernel_spmd

F32 = mybir.dt.float32
BF16 = mybir.dt.bfloat16
AF = mybir.ActivationFunctionType
ALU = mybir.AluOpType

S_FULL = 512
B_FULL = 256
F = 128          # seq features
H = 256          # hidden (both LSTMs)
NQ = 8           # 4*H / 128 gate m-tiles
NCORES = 8
BL = B_FULL // NCORES  # 32 batch per core
TSEG = 8         # scan steps per bulk segment
TMLP = 16        # time steps per MLP chunk

# torch gate order in the 4H rows: (i, f, g, o), 256 rows each.
_i, _f, _g, _o = np.r_[0:256], np.r_[256:512], np.r_[512:768], np.r_[768:1024]
# on-chip q-tile order per hidden half: (g, f, i, o) -- matches the state
# tile slot order [cs, tg, tf, ti, to] written by one strided tanh.
GATE_PERM = np.concatenate([
    _g[:128], _f[:128], _i[:128], _o[:128],
    _g[128:], _f[128:], _i[128:], _o[128:],
])
def _row_scale() -> np.ndarray:
    """Per-permuted-row scale: 1.0 for g rows, 0.5 for f/i/o rows."""
    s = np.empty(1024, np.float32)
    for h in range(2):
        base = 512 * h
        s[base:base + 128] = 1.0          # g
        s[base + 128:base + 512] = 0.5    # f, i, o
    return s


ROW_SCALE = _row_scale()


# --------------------------------------------------------------------------
# host-side preparation
# --------------------------------------------------------------------------

def prep_weights(inp: dict) -> dict:
    """Gate-permute, scale and transpose all weights. Shared across cores."""
    f32 = lambda x: np.asarray(x, np.float32)

    def gates(w, bias, col_scale_rows=None):
        """w: [1024, IN]; returns ([IN+1, 1024]) with bias as last row."""
        p = f32(w)[GATE_PERM] * ROW_SCALE[:, None]
        b = f32(bias)[GATE_PERM] * ROW_SCALE
        wt = np.concatenate([p.T, b[None, :]], axis=0)  # [IN+1, 1024]
        return wt

    out = {}
    bc = f32(inp["bih_c"]) + f32(inp["bhh_c"])
    bg = f32(inp["bih_g"]) + f32(inp["bhh_g"])

    # constraint input weights [130, 1024]: 129 features + bias row
    out["wihc"] = gates(inp["Wih_c"], bc).astype(bfloat16)
    # constraint recurrent [256, 1024], consumes H2 -> extra 0.5
    whhc = (f32(inp["Whh_c"])[GATE_PERM] * ROW_SCALE[:, None]).T * 0.5
    out["whhc"] = np.ascontiguousarray(whhc).astype(bfloat16)

    wg = f32(inp["Wih_g"])[GATE_PERM] * ROW_SCALE[:, None]   # [1024, 384]
    # gen x-part [129, 1024]: 128 features + bias row
    out["wgx"] = np.concatenate(
        [wg[:, :F].T, (bg[GATE_PERM] * ROW_SCALE)[None, :]], axis=0
    ).astype(bfloat16)
    # gen hc-part [256, 1024], consumes H2c -> extra 0.5
    out["wghc"] = np.ascontiguousarray(wg[:, F:].T * 0.5).astype(bfloat16)
    whhg = (f32(inp["Whh_g"])[GATE_PERM] * ROW_SCALE[:, None]).T * 0.5
    out["whhg"] = np.ascontiguousarray(whhg).astype(bfloat16)

    # MLP head; W1 consumes H2g -> 0.5
    out["w1t"] = np.ascontiguousarray(f32(inp["W1"]).T * 0.5).astype(bfloat16)
    out["w2t"] = np.ascontiguousarray(f32(inp["W2"]).T).astype(bfloat16)
    out["b1"] = np.ascontiguousarray(f32(inp["b1"])[:, None])
    out["b2"] = np.ascontiguousarray(f32(inp["b2"])[:, None])
    return out


def prep_core_inputs(seq, seq_constraints, c0, c1, s):
    """Per-core activation tensors for batch columns [c0:c1), seq len s."""
    bl = c1 - c0
    xc = np.asarray(seq_constraints, np.float32)[:s, c0:c1]   # [s, bl, 129]
    # time-reversed + transposed: xcT[k, tau, b] = xc[s-1-tau, b, k]
    xcT = np.empty((130, s, bl), np.float32)
    xcT[:129] = xc[::-1].transpose(2, 0, 1)
    xcT[129] = 1.0                                            # bias ones-row
    sq = np.asarray(seq, np.float32)[:s, c0:c1]               # [s, bl, 128]
    xgT = np.empty((129, s, bl), np.float32)
    xgT[0:128, 0] = 0.0
    xgT[0:128, 1:] = sq[:-1].transpose(2, 0, 1)
    xgT[128] = 1.0
    return {"xcT": xcT.astype(bfloat16), "xgT": xgT.astype(bfloat16)}


# --------------------------------------------------------------------------
# device program
# --------------------------------------------------------------------------

def build_program(s=S_FULL, tseg=TSEG, bl=BL):
    """Build + compile the per-core Bass program. Returns (nc, out_name)."""
    assert s % tseg == 0 and s % TMLP == 0
    nseg = s // tseg
    nc = bacc.Bacc("TRN2", target_bir_lowering=False, debug=False,
                   enable_asserts=False)

    d_xcT = nc.dram_tensor("xcT", [130, s, bl], BF16, kind="ExternalInput")
    d_xgT = nc.dram_tensor("xgT", [129, s, bl], BF16, kind="ExternalInput")
    d_wihc = nc.dram_tensor("wihc", [130, 4 * H], BF16, kind="ExternalInput")
    d_whhc = nc.dram_tensor("whhc", [H, 4 * H], BF16, kind="ExternalInput")
    d_wgx = nc.dram_tensor("wgx", [129, 4 * H], BF16, kind="ExternalInput")
    d_wghc = nc.dram_tensor("wghc", [H, 4 * H], BF16, kind="ExternalInput")
    d_whhg = nc.dram_tensor("whhg", [H, 4 * H], BF16, kind="ExternalInput")
    d_w1t = nc.dram_tensor("w1t", [H, F], BF16, kind="ExternalInput")
    d_w2t = nc.dram_tensor("w2t", [F, F], BF16, kind="ExternalInput")
    d_b1 = nc.dram_tensor("b1", [128, 1], F32, kind="ExternalInput")
    d_b2 = nc.dram_tensor("b2", [128, 1], F32, kind="ExternalInput")
    d_out = nc.dram_tensor("out", [F, s, bl], F32, kind="ExternalOutput")

    with tile.TileContext(nc) as tc, ExitStack() as ctx:
        wp = ctx.enter_context(tc.tile_pool(name="weights", bufs=1))
        hcp = ctx.enter_context(tc.tile_pool(name="hstore", bufs=1))
        xinp = ctx.enter_context(tc.tile_pool(name="xin", bufs=3))
        stp = ctx.enter_context(tc.tile_pool(name="state", bufs=4))
        vup = ctx.enter_context(tc.tile_pool(name="vu", bufs=3))
        tcp = ctx.enter_context(tc.tile_pool(name="tcell", bufs=3))
        yp = ctx.enter_context(tc.tile_pool(name="yout", bufs=3))

        # ---- load weights to SBUF (resident all kernel) ----
        def wtile(dram, shape, dt=BF16, row0=0, tag=None):
            t = wp.tile(shape, dt, tag=tag or f"w_{dram.name}_{row0}",
                         name=f"w_{dram.name}_{row0}")
            nc.sync.dma_start(t[:], dram.ap()[row0:row0 + shape[0]])
            return t

        wihc0 = wtile(d_wihc, [128, 4 * H])
        wihc1 = wtile(d_wihc, [2, 4 * H], row0=128)
        whhc = [wtile(d_whhc, [128, 4 * H], row0=128 * k) for k in range(2)]
        wgx0 = wtile(d_wgx, [128, 4 * H])
        wgx1 = wtile(d_wgx, [1, 4 * H], row0=128)
        wghc = [wtile(d_wghc, [128, 4 * H], row0=128 * k) for k in range(2)]
        whhg = [wtile(d_whhg, [128, 4 * H], row0=128 * k) for k in range(2)]
        w1t = [wtile(d_w1t, [128, F], row0=128 * k) for k in range(2)]
        w2t = wtile(d_w2t, [128, F])
        b1_sb = wtile(d_b1, [128, 1], F32)
        b2_sb = wtile(d_b2, [128, 1], F32)

        # hidden stores (H2 = 2*h, bf16), per hidden half
        hc = [hcp.tile([128, s, bl], BF16, tag=f"hc{k}", name=f"hc{k}")
              for k in range(2)]
        hg = [hcp.tile([128, s, bl], BF16, tag=f"hg{k}", name=f"hg{k}")
              for k in range(2)]

        # zero h for step 0
        hz = hcp.tile([128, bl], BF16, tag="hz", name="hz")
        nc.vector.memset(hz[:], 0.0)

        def scan_phase(psb, d_x0, d_x1, x1_rows, wih0, wih1, whh, hstore,
                       reverse, hc_bulk):
            """One LSTM scan over the full sequence."""
            h_prev = [hz[:], hz[:]]
            st_cur = []
            for h in range(2):
                st = stp.tile([128, 5, bl], F32, tag=f"st{h}", name=f"st{h}")
                nc.vector.memset(st[:, 0, :], 0.0)      # cs_0 = 0
                st_cur.append(st)

            for seg in range(nseg):
                t0 = seg * tseg
                x0 = xinp.tile([128, tseg, bl], BF16, tag="x0", name="x0")
                nc.sync.dma_start(x0[:], d_x0.ap()[0:128, t0:t0 + tseg])
                x1 = xinp.tile([x1_rows, tseg, bl], BF16, tag="x1", name="x1")
                nc.sync.dma_start(x1[:], d_x1.ap()[128:128 + x1_rows,
                                                   t0:t0 + tseg])
                psA = psb.tile([128, NQ, tseg, bl], F32, tag="psA", name="psA")
                # bulk input projection (+ ones-row bias) into PSUM
                for q in range(NQ):
                    c = 128 * q
                    nc.tensor.matmul(psA[:, q], wih0[:, c:c + 128], x0[:],
                                     start=(q % 2 == 0), stop=False,
                                     skip_group_check=True)
                    nc.tensor.matmul(psA[:, q], wih1[:, c:c + 128], x1[:],
                                     start=False, stop=False,
                                     skip_group_check=True)
                    if hc_bulk is not None:
                        for k in range(2):
                            nc.tensor.matmul(
                                psA[:, q], hc_bulk[1][k][:, c:c + 128],
                                hc_bulk[0][k][:, t0:t0 + tseg],
                                start=False, stop=False,
                                skip_group_check=True)

                for tl in range(tseg):
                    t = t0 + tl
                    t_out = (s - 1 - t) if reverse else t
                    hp = list(h_prev)
                    st_next = [stp.tile([128, 5, bl], F32, tag=f"st{h}",
                                        name=f"stn{h}")
                               for h in range(2)]
                    for h in range(2):
                        for k in range(2):
                            for q in range(4 * h, 4 * h + 4):
                                c = 128 * q
                                nc.tensor.matmul(
                                    psA[:, q, tl], whh[k][:, c:c + 128],
                                    hp[k], start=False, stop=(k == 1),
                                    skip_group_check=True)
                        st = st_cur[h]
                        # [tg, tf, ti, to] <- tanh(gate pre-acts)
                        nc.scalar.activation(st[:, 1:5],
                                             psA[:, 4 * h:4 * h + 4, tl],
                                             AF.Tanh)
                        vu = vup.tile([128, 2, bl], F32, tag=f"vu{h}", name=f"vu{h}")
                        # v2 = (tf+1)*cs ; u2 = (ti+1)*tg
                        nc.vector.scalar_tensor_tensor(
                            vu[:], st[:, 2:4], 1.0, st[:, 0:2],
                            ALU.add, ALU.mult)
                        # cs_new = v2*0.5 + u2
                        nc.vector.scalar_tensor_tensor(
                            st_next[h][:, 0], vu[:, 0], 0.5, vu[:, 1],
                            ALU.mult, ALU.add)
                        tcl = tcp.tile([128, bl], F32, tag=f"tc{h}", name=f"tc{h}")
                        nc.scalar.activation(tcl[:], st_next[h][:, 0],
                                             AF.Tanh, scale=0.5)
                        # H2 = (to+1)*tc -> bf16 hidden store
                        nc.vector.scalar_tensor_tensor(
                            hstore[h][:, t_out], st[:, 4], 1.0, tcl[:],
                            ALU.add, ALU.mult)
                        h_prev[h] = hstore[h][:, t_out]
                    st_cur = st_next

        with tc.tile_pool(name="psscan", bufs=2, space="PSUM") as psb:
            # phase C: constraint LSTM, backward in time
            scan_phase(psb, d_xcT, d_xcT, 2, wihc0, wihc1, whhc, hc,
                       reverse=True, hc_bulk=None)
            # phase G: generation LSTM, forward
            scan_phase(psb, d_xgT, d_xgT, 1, wgx0, wgx1, whhg, hg,
                       reverse=False, hc_bulk=(hc, wghc))

        # ---- phase M: bulk MLP head over all stored hg ----
        with tc.tile_pool(name="psmlp", bufs=4, space="PSUM") as psm:
            for t0 in range(0, s, TMLP):
                ps1 = psm.tile([128, TMLP, bl], F32, tag="ps1", name="ps1")
                for k in range(2):
                    nc.tensor.matmul(ps1[:], w1t[k][:],
                                     hg[k][:, t0:t0 + TMLP],
                                     start=(k == 0), stop=(k == 1))
                y1 = yp.tile([128, TMLP, bl], BF16, tag="y1", name="y1")
                nc.scalar.activation(y1[:], ps1[:], AF.Relu,
                                     bias=b1_sb[:, 0:1])
                ps2 = psm.tile([128, TMLP, bl], F32, tag="ps2", name="ps2")
                nc.tensor.matmul(ps2[:], w2t[:], y1[:], start=True, stop=True)
                y2 = yp.tile([128, TMLP, bl], F32, tag="y2", name="y2")
                nc.scalar.activation(y2[:], ps2[:], AF.Identity,
                                     bias=b2_sb[:, 0:1])
                nc.sync.dma_start(d_out.ap()[:, t0:t0 + TMLP], y2[:])

    nc.compile()
    return nc, "out"


_PROGRAM_CACHE = {}


def get_program(s=S_FULL, tseg=TSEG, bl=BL):
    key = (s, tseg, bl)
    if key not in _PROGRAM_CACHE:
        _PROGRAM_CACHE[key] = build_program(s, tseg, bl)
    return _PROGRAM_CACHE[key]


# --------------------------------------------------------------------------
# entry point
# --------------------------------------------------------------------------

def kernel(**inputs) -> np.ndarray:
    s, b = np.asarray(inputs["seq"]).shape[:2]
    assert (s, b) == (S_FULL, B_FULL)
    nc, out_name = get_program()
    w = prep_weights(inputs)
    in_maps = []
    for core in range(NCORES):
        c0 = core * BL
        m = dict(w)
        m.update(prep_core_inputs(inputs["seq"], inputs["seq_constraints"],
                                  c0, c0 + BL, S_FULL))
        in_maps.append(m)
    res = run_bass_kernel_spmd(nc, in_maps, core_ids=list(range(NCORES)))
    # per-core out: [F, S, BL] -> [S, BL, F]; concat cores along batch
    parts = [np.transpose(res.results[c][out_name], (1, 2, 0))
             for c in range(NCORES)]
    return np.ascontiguousarray(np.concatenate(parts, axis=1))


# revision 10
# speedup vs baseline: 2.1985x; 1.3758x over previous
"""Trainium2 Bass kernel for nn_ConstraintModel (2-LSTM chain + MLP head).

Contract: kernel(**inputs) takes FULL unsharded inputs (numpy, keyed as in
setup_inputs()) and returns the FULL (512, 256, 128) float32 output.

Strategy: data-parallel over batch (256 -> 8 cores x 32). Each core runs an
identical Bass program on its batch shard:
  phase C: constraint LSTM scanned backward over the 512 steps
  phase G: generation LSTM scanned forward, consuming the stored constraint
           hiddens
  phase M: bulk MLP head over all stored generation hiddens.

Layout: [feature/hidden on SBUF partitions, batch on the free dim] so the
recurrent matmuls produce gates.T directly and elementwise gate math runs on
all 128 partitions.

Key structure (v2):
- The per-segment input projections (x @ Wih + bias) are accumulated DIRECTLY
  into the per-step gate PSUM regions; the per-step recurrent matmuls then
  accumulate on top (start=False).  No per-step gate add, no PSUM->SBUF
  copies.  Biases ride along as an extra ones-row of the input.
- Recurrent matmul: 16 [128c x 128p x 32f] matmuls per step (2 contraction
  halves x 8 gate tiles), bf16 weights (FWL-eligible 128-col stationaries).
- Gate nonlinearity: ONE tanh activation per hidden half covering all 4 gate
  tiles, using sigmoid(x) = (tanh(x/2)+1)/2.  The 1/2 pre-scale is folded
  into the i/f/o weight rows; the (t+1)/2 fix-ups are folded into fused
  scalar_tensor_tensor ops:
      v2 = (tf + 1) * cs_prev          # = 4*sigm(f)*c_prev   (cs = 2c)
      u2 = (ti + 1) * tg               # = 2*sigm(i)*tanh(g)
      cs = v2*0.5 + u2                 # = 2*c_new
      tc = tanh(0.5 * cs)              # = tanh(c_new)
      H2 = (to + 1) * tc               # = 2*h_new
  h is stored scaled by 2 (bf16); every weight column that consumes h is
  pre-scaled by 0.5 on the host, making the convention exact.
"""

import sys
from contextlib import ExitStack

sys.path.insert(0, "/opt/pypackages")
sys.path.insert(0, "/opt/trn_rl_repo")

import numpy as np
from ml_dtypes import bfloat16

import concourse.bass as bass
import concourse.bacc as bacc
import concourse.tile as tile
from concourse import mybir
from concourse.bass_utils import run_bass_kernel_spmd

F32 = mybir.dt.float32
BF16 = mybir.dt.bfloat16
AF = mybir.ActivationFunctionType
ALU = mybir.AluOpType

S_FULL = 512
B_FULL = 256
F = 128          # seq features
H = 256          # hidden (both LSTMs)
NQ = 8           # 4*H / 128 gate m-tiles
NCORES = 8
BL = B_FULL // NCORES  # 32 batch per core
TSEG = 8         # scan steps per bulk segment
TMLP = 16        # time steps per MLP chunk

# torch gate order in the 4H rows: (i, f, g, o), 256 rows each.
_i, _f, _g, _o = np.r_[0:256], np.r_[256:512], np.r_[512:768], np.r_[768:1024]
# on-chip q-tile order per hidden half: (g, f, i, o) -- matches the state
# tile slot order [cs, tg, tf, ti, to] written by one strided tanh.
GATE_PERM = np.concatenate([
    _g[:128], _f[:128], _i[:128], _o[:128],
    _g[128:], _f[128:], _i[128:], _o[128:],
])
def _row_scale() -> np.ndarray:
    """Per-permuted-row scale: 1.0 for g rows, 0.5 for f/i/o rows."""
    s = np.empty(1024, np.float32)
    for h in range(2):
        base = 512 * h
        s[base:base + 128] = 1.0          # g
        s[base + 128:base + 512] = 0.5    # f, i, o
    return s


ROW_SCALE = _row_scale()


# --------------------------------------------------------------------------
# host-side preparation
# --------------------------------------------------------------------------

def prep_weights(inp: dict) -> dict:
    """Gate-permute, scale and transpose all weights. Shared across cores."""
    f32 = lambda x: np.asarray(x, np.float32)

    def gates(w, bias, col_scale_rows=None):
        """w: [1024, IN]; returns ([IN+1, 1024]) with bias as last row."""
        p = f32(w)[GATE_PERM] * ROW_SCALE[:, None]
        b = f32(bias)[GATE_PERM] * ROW_SCALE
        wt = np.concatenate([p.T, b[None, :]], axis=0)  # [IN+1, 1024]
        return wt

    out = {}
    bc = f32(inp["bih_c"]) + f32(inp["bhh_c"])
    bg = f32(inp["bih_g"]) + f32(inp["bhh_g"])

    def pad256(wt):
        """Zero-pad [IN+1, 1024] to [256, 1024] so the second contraction
        tile is a full 128 rows (FWL-eligible LDWEIGHTS; the zero rows make
        any garbage in the padded moving-operand rows contribute 0)."""
        p = np.zeros((256, 1024), np.float32)
        p[:wt.shape[0]] = wt
        return p

    # constraint input weights: 129 features + bias row, padded to 256
    out["wihc"] = pad256(gates(inp["Wih_c"], bc)).astype(bfloat16)
    # constraint recurrent [256, 1024], consumes H2 -> extra 0.5
    whhc = (f32(inp["Whh_c"])[GATE_PERM] * ROW_SCALE[:, None]).T * 0.5
    out["whhc"] = np.ascontiguousarray(whhc).astype(bfloat16)

    wg = f32(inp["Wih_g"])[GATE_PERM] * ROW_SCALE[:, None]   # [1024, 384]
    # gen x-part: 128 features + bias row, padded to 256
    out["wgx"] = pad256(np.concatenate(
        [wg[:, :F].T, (bg[GATE_PERM] * ROW_SCALE)[None, :]], axis=0
    )).astype(bfloat16)
    # gen hc-part [256, 1024], consumes H2c -> extra 0.5
    out["wghc"] = np.ascontiguousarray(wg[:, F:].T * 0.5).astype(bfloat16)
    whhg = (f32(inp["Whh_g"])[GATE_PERM] * ROW_SCALE[:, None]).T * 0.5
    out["whhg"] = np.ascontiguousarray(whhg).astype(bfloat16)

    # MLP head; W1 consumes H2g -> 0.5
    out["w1t"] = np.ascontiguousarray(f32(inp["W1"]).T * 0.5).astype(bfloat16)
    out["w2t"] = np.ascontiguousarray(f32(inp["W2"]).T).astype(bfloat16)
    out["b1"] = np.ascontiguousarray(f32(inp["b1"])[:, None])
    out["b2"] = np.ascontiguousarray(f32(inp["b2"])[:, None])
    return out


def prep_core_inputs(seq, seq_constraints, c0, c1, s):
    """Per-core activation tensors for batch columns [c0:c1), seq len s."""
    bl = c1 - c0
    xc = np.asarray(seq_constraints, np.float32)[:s, c0:c1]   # [s, bl, 129]
    # time-reversed + transposed: xcT[k, tau, b] = xc[s-1-tau, b, k]
    xcT = np.empty((130, s, bl), np.float32)
    xcT[:129] = xc[::-1].transpose(2, 0, 1)
    xcT[129] = 1.0                                            # bias ones-row
    sq = np.asarray(seq, np.float32)[:s, c0:c1]               # [s, bl, 128]
    xgT = np.empty((129, s, bl), np.float32)
    xgT[0:128, 0] = 0.0
    xgT[0:128, 1:] = sq[:-1].transpose(2, 0, 1)
    xgT[128] = 1.0
    return {"xcT": xcT.astype(bfloat16), "xgT": xgT.astype(bfloat16)}


# --------------------------------------------------------------------------
# device program
# --------------------------------------------------------------------------

def build_program(s=S_FULL, tseg=TSEG, bl=BL):
    """Build + compile the per-core Bass program. Returns (nc, out_name)."""
    assert s % tseg == 0 and s % TMLP == 0
    nseg = s // tseg
    nc = bacc.Bacc("TRN2", target_bir_lowering=False, debug=False,
                   enable_asserts=False)

    d_xcT = nc.dram_tensor("xcT", [130, s, bl], BF16, kind="ExternalInput")
    d_xgT = nc.dram_tensor("xgT", [129, s, bl], BF16, kind="ExternalInput")
    d_wihc = nc.dram_tensor("wihc", [256, 4 * H], BF16, kind="ExternalInput")
    d_whhc = nc.dram_tensor("whhc", [H, 4 * H], BF16, kind="ExternalInput")
    d_wgx = nc.dram_tensor("wgx", [256, 4 * H], BF16, kind="ExternalInput")
    d_wghc = nc.dram_tensor("wghc", [H, 4 * H], BF16, kind="ExternalInput")
    d_whhg = nc.dram_tensor("whhg", [H, 4 * H], BF16, kind="ExternalInput")
    d_w1t = nc.dram_tensor("w1t", [H, F], BF16, kind="ExternalInput")
    d_w2t = nc.dram_tensor("w2t", [F, F], BF16, kind="ExternalInput")
    d_b1 = nc.dram_tensor("b1", [128, 1], F32, kind="ExternalInput")
    d_b2 = nc.dram_tensor("b2", [128, 1], F32, kind="ExternalInput")
    d_out = nc.dram_tensor("out", [F, s, bl], F32, kind="ExternalOutput")

    with tile.TileContext(nc) as tc, ExitStack() as ctx:
        wp = ctx.enter_context(tc.tile_pool(name="weights", bufs=1))
        hcp = ctx.enter_context(tc.tile_pool(name="hstore", bufs=1))
        xinp = ctx.enter_context(tc.tile_pool(name="xin", bufs=3))
        stp = ctx.enter_context(tc.tile_pool(name="state", bufs=4))
        vup = ctx.enter_context(tc.tile_pool(name="vu", bufs=3))
        tcp = ctx.enter_context(tc.tile_pool(name="tcell", bufs=3))
        yp = ctx.enter_context(tc.tile_pool(name="yout", bufs=3))

        # ---- load weights to SBUF (resident all kernel) ----
        def wtile(dram, shape, dt=BF16, row0=0, tag=None):
            t = wp.tile(shape, dt, tag=tag or f"w_{dram.name}_{row0}",
                         name=f"w_{dram.name}_{row0}")
            nc.sync.dma_start(t[:], dram.ap()[row0:row0 + shape[0]])
            return t

        wihc0 = wtile(d_wihc, [128, 4 * H])
        wihc1 = wtile(d_wihc, [128, 4 * H], row0=128)
        whhc = [wtile(d_whhc, [128, 4 * H], row0=128 * k) for k in range(2)]
        wgx0 = wtile(d_wgx, [128, 4 * H])
        wgx1 = wtile(d_wgx, [128, 4 * H], row0=128)
        wghc = [wtile(d_wghc, [128, 4 * H], row0=128 * k) for k in range(2)]
        whhg = [wtile(d_whhg, [128, 4 * H], row0=128 * k) for k in range(2)]
        w1t = [wtile(d_w1t, [128, F], row0=128 * k) for k in range(2)]
        w2t = wtile(d_w2t, [128, F])
        b1_sb = wtile(d_b1, [128, 1], F32)
        b2_sb = wtile(d_b2, [128, 1], F32)

        # hidden stores (H2 = 2*h, bf16), per hidden half
        hc = [hcp.tile([128, s, bl], BF16, tag=f"hc{k}", name=f"hc{k}")
              for k in range(2)]
        hg = [hcp.tile([128, s, bl], BF16, tag=f"hg{k}", name=f"hg{k}")
              for k in range(2)]

        # zero h for step 0
        hz = hcp.tile([128, bl], BF16, tag="hz", name="hz")
        nc.vector.memset(hz[:], 0.0)

        # padded second-contraction-tile inputs (rows >= x1_rows stay 0 from
        # the one-time memset; the matching weight rows are 0 anyway, the
        # zeroing just guards against NaN garbage)
        x1t = [hcp.tile([128, tseg, bl], BF16, tag=f"x1_{i}", name=f"x1_{i}")
               for i in range(2)]
        for t_ in x1t:
            nc.vector.memset(t_[:], 0.0)

        def scan_phase(psb, d_x, x1_rows, wih0, wih1, whh, hstore,
                       reverse, hc_bulk):
            """One LSTM scan over the full sequence."""

            def seg_tiles(seg):
                """DMA inputs + allocate gate PSUM for a segment; return the
                (not yet emitted) bulk matmul argument list."""
                t0 = seg * tseg
                x0 = xinp.tile([128, tseg, bl], BF16, tag="x0", name="x0")
                nc.sync.dma_start(x0[:], d_x.ap()[0:128, t0:t0 + tseg])
                x1 = x1t[seg % 2]
                nc.sync.dma_start(x1[0:x1_rows],
                                  d_x.ap()[128:128 + x1_rows, t0:t0 + tseg])
                psA = [psb.tile([128, 4, tseg, bl], F32, tag=f"psA{h}",
                                name=f"psA{h}") for h in range(2)]
                mms = []
                for h in range(2):
                    for qq in range(4):
                        c = 128 * (4 * h + qq)
                        # start=True on the first write to each PSUM bank
                        mms.append((psA[h][:, qq], wih0[:, c:c + 128], x0[:],
                                    qq % 2 == 0))
                        mms.append((psA[h][:, qq], wih1[:, c:c + 128], x1[:],
                                    False))
                        if hc_bulk is not None:
                            for k in range(2):
                                mms.append((psA[h][:, qq],
                                            hc_bulk[1][k][:, c:c + 128],
                                            hc_bulk[0][k][:, t0:t0 + tseg],
                                            False))
                return psA, mms

            def emit_bulk(mms):
                for out, lhsT, rhs, start in mms:
                    nc.tensor.matmul(out, lhsT, rhs, start=start, stop=False,
                                     skip_group_check=True)

            h_prev = [hz[:], hz[:]]
            st_cur = []
            for h in range(2):
                st = stp.tile([128, 5, bl], F32, tag=f"st{h}", name=f"st{h}")
                nc.vector.memset(st[:, 0, :], 0.0)      # cs_0 = 0
                st_cur.append(st)

            psA, mms = seg_tiles(0)
            emit_bulk(mms)
            for seg in range(nseg):
                if seg + 1 < nseg:
                    psA_n, mms_n = seg_tiles(seg + 1)
                else:
                    psA_n, mms_n = None, []
                # next segment's bulk matmuls, interleaved into this
                # segment's steps to fill tensor-engine idle time
                chunk = -(-len(mms_n) // tseg) if mms_n else 0

                for tl in range(tseg):
                    t = seg * tseg + tl
                    t_out = (s - 1 - t) if reverse else t
                    hp = list(h_prev)
                    st_next = [stp.tile([128, 5, bl], F32, tag=f"st{h}",
                                        name=f"stn{h}")
                               for h in range(2)]
                    for h in range(2):
                        for k in range(2):
                            for qq in range(4):
                                c = 128 * (4 * h + qq)
                                nc.tensor.matmul(
                                    psA[h][:, qq, tl], whh[k][:, c:c + 128],
                                    hp[k], start=False, stop=(k == 1),
                                    skip_group_check=True)
                    emit_bulk(mms_n[tl * chunk:(tl + 1) * chunk])
                    for h in range(2):
                        st = st_cur[h]
                        # [tg, tf, ti, to] <- tanh(gate pre-acts)
                        nc.scalar.activation(st[:, 1:5], psA[h][:, :, tl],
                                             AF.Tanh)
                        vu = vup.tile([128, 2, bl], F32, tag=f"vu{h}",
                                      name=f"vu{h}")
                        # v2 = (tf+1)*cs ; u2 = (ti+1)*tg
                        nc.vector.scalar_tensor_tensor(
                            vu[:], st[:, 2:4], 1.0, st[:, 0:2],
                            ALU.add, ALU.mult)
                        # cs_new = v2*0.5 + u2
                        nc.vector.scalar_tensor_tensor(
                            st_next[h][:, 0], vu[:, 0], 0.5, vu[:, 1],
                            ALU.mult, ALU.add)
                        tcl = tcp.tile([128, bl], F32, tag=f"tc{h}",
                                       name=f"tc{h}")
                        nc.scalar.activation(tcl[:], st_next[h][:, 0],
                                             AF.Tanh, scale=0.5)
                        # H2 = (to+1)*tc -> bf16 hidden store
                        nc.vector.scalar_tensor_tensor(
                            hstore[h][:, t_out], st[:, 4], 1.0, tcl[:],
                            ALU.add, ALU.mult)
                        h_prev[h] = hstore[h][:, t_out]
                    st_cur = st_next
                psA = psA_n

        with tc.tile_pool(name="psscan", bufs=2, space="PSUM") as psb:
            # phase C: constraint LSTM, backward in time
            scan_phase(psb, d_xcT, 2, wihc0, wihc1, whhc, hc,
                       reverse=True, hc_bulk=None)
            # phase G: generation LSTM, forward
            scan_phase(psb, d_xgT, 1, wgx0, wgx1, whhg, hg,
                       reverse=False, hc_bulk=(hc, wghc))

        # ---- phase M: bulk MLP head over all stored hg ----
        with tc.tile_pool(name="psmlp", bufs=4, space="PSUM") as psm:
            for t0 in range(0, s, TMLP):
                ps1 = psm.tile([128, TMLP, bl], F32, tag="ps1", name="ps1")
                for k in range(2):
                    nc.tensor.matmul(ps1[:], w1t[k][:],
                                     hg[k][:, t0:t0 + TMLP],
                                     start=(k == 0), stop=(k == 1))
                y1 = yp.tile([128, TMLP, bl], BF16, tag="y1", name="y1")
                nc.scalar.activation(y1[:], ps1[:], AF.Relu,
                                     bias=b1_sb[:, 0:1])
                ps2 = psm.tile([128, TMLP, bl], F32, tag="ps2", name="ps2")
                nc.tensor.matmul(ps2[:], w2t[:], y1[:], start=True, stop=True)
                y2 = yp.tile([128, TMLP, bl], F32, tag="y2", name="y2")
                nc.scalar.activation(y2[:], ps2[:], AF.Identity,
                                     bias=b2_sb[:, 0:1])
                nc.sync.dma_start(d_out.ap()[:, t0:t0 + TMLP], y2[:])

    nc.compile()
    return nc, "out"


_PROGRAM_CACHE = {}


def get_program(s=S_FULL, tseg=TSEG, bl=BL):
    key = (s, tseg, bl)
    if key not in _PROGRAM_CACHE:
        _PROGRAM_CACHE[key] = build_program(s, tseg, bl)
    return _PROGRAM_CACHE[key]


# --------------------------------------------------------------------------
# entry point
# --------------------------------------------------------------------------

def kernel(**inputs) -> np.ndarray:
    s, b = np.asarray(inputs["seq"]).shape[:2]
    assert (s, b) == (S_FULL, B_FULL)
    nc, out_name = get_program()
    w = prep_weights(inputs)
    in_maps = []
    for core in range(NCORES):
        c0 = core * BL
        m = dict(w)
        m.update(prep_core_inputs(inputs["seq"], inputs["seq_constraints"],
                                  c0, c0 + BL, S_FULL))
        in_maps.append(m)
    res = run_bass_kernel_spmd(nc, in_maps, core_ids=list(range(NCORES)))
    # per-core out: [F, S, BL] -> [S, BL, F]; concat cores along batch
    parts = [np.transpose(res.results[c][out_name], (1, 2, 0))
             for c in range(NCORES)]
    return np.ascontiguousarray(np.concatenate(parts, axis=1))


# revision 11
# speedup vs baseline: 2.2007x; 1.0010x over previous
"""Trainium2 Bass kernel for nn_ConstraintModel (2-LSTM chain + MLP head).

Contract: kernel(**inputs) takes FULL unsharded inputs (numpy, keyed as in
setup_inputs()) and returns the FULL (512, 256, 128) float32 output.

Strategy: data-parallel over batch (256 -> 8 cores x 32). Each core runs an
identical Bass program on its batch shard:
  phase C: constraint LSTM scanned backward over the 512 steps
  phase G: generation LSTM scanned forward, consuming the stored constraint
           hiddens
  phase M: bulk MLP head over all stored generation hiddens.

Layout: [feature/hidden on SBUF partitions, batch on the free dim] so the
recurrent matmuls produce gates.T directly and elementwise gate math runs on
all 128 partitions.

Key structure (v2):
- The per-segment input projections (x @ Wih + bias) are accumulated DIRECTLY
  into the per-step gate PSUM regions; the per-step recurrent matmuls then
  accumulate on top (start=False).  No per-step gate add, no PSUM->SBUF
  copies.  Biases ride along as an extra ones-row of the input.
- Recurrent matmul: 16 [128c x 128p x 32f] matmuls per step (2 contraction
  halves x 8 gate tiles), bf16 weights (FWL-eligible 128-col stationaries).
- Gate nonlinearity: ONE tanh activation per hidden half covering all 4 gate
  tiles, using sigmoid(x) = (tanh(x/2)+1)/2.  The 1/2 pre-scale is folded
  into the i/f/o weight rows; the (t+1)/2 fix-ups are folded into fused
  scalar_tensor_tensor ops:
      v2 = (tf + 1) * cs_prev          # = 4*sigm(f)*c_prev   (cs = 2c)
      u2 = (ti + 1) * tg               # = 2*sigm(i)*tanh(g)
      cs = v2*0.5 + u2                 # = 2*c_new
      tc = tanh(0.5 * cs)              # = tanh(c_new)
      H2 = (to + 1) * tc               # = 2*h_new
  h is stored scaled by 2 (bf16); every weight column that consumes h is
  pre-scaled by 0.5 on the host, making the convention exact.
"""

import sys
from contextlib import ExitStack

sys.path.insert(0, "/opt/pypackages")
sys.path.insert(0, "/opt/trn_rl_repo")

import numpy as np
from ml_dtypes import bfloat16

import concourse.bass as bass
import concourse.bacc as bacc
import concourse.tile as tile
from concourse import mybir
from concourse.bass_utils import run_bass_kernel_spmd

F32 = mybir.dt.float32
BF16 = mybir.dt.bfloat16
AF = mybir.ActivationFunctionType
ALU = mybir.AluOpType

S_FULL = 512
B_FULL = 256
F = 128          # seq features
H = 256          # hidden (both LSTMs)
NQ = 8           # 4*H / 128 gate m-tiles
NCORES = 8
BL = B_FULL // NCORES  # 32 batch per core
TSEG = 8         # scan steps per bulk segment
TMLP = 16        # time steps per MLP chunk

# torch gate order in the 4H rows: (i, f, g, o), 256 rows each.
_i, _f, _g, _o = np.r_[0:256], np.r_[256:512], np.r_[512:768], np.r_[768:1024]
# on-chip q-tile order per hidden half: (g, f, i, o) -- matches the state
# tile slot order [cs, tg, tf, ti, to] written by one strided tanh.
GATE_PERM = np.concatenate([
    _g[:128], _f[:128], _i[:128], _o[:128],
    _g[128:], _f[128:], _i[128:], _o[128:],
])
def _row_scale() -> np.ndarray:
    """Per-permuted-row scale: 1.0 for g rows, 0.5 for f/i/o rows."""
    s = np.empty(1024, np.float32)
    for h in range(2):
        base = 512 * h
        s[base:base + 128] = 1.0          # g
        s[base + 128:base + 512] = 0.5    # f, i, o
    return s


ROW_SCALE = _row_scale()


# --------------------------------------------------------------------------
# host-side preparation
# --------------------------------------------------------------------------

def prep_weights(inp: dict) -> dict:
    """Gate-permute, scale and transpose all weights. Shared across cores."""
    f32 = lambda x: np.asarray(x, np.float32)

    def gates(w, bias, col_scale_rows=None):
        """w: [1024, IN]; returns ([IN+1, 1024]) with bias as last row."""
        p = f32(w)[GATE_PERM] * ROW_SCALE[:, None]
        b = f32(bias)[GATE_PERM] * ROW_SCALE
        wt = np.concatenate([p.T, b[None, :]], axis=0)  # [IN+1, 1024]
        return wt

    out = {}
    bc = f32(inp["bih_c"]) + f32(inp["bhh_c"])
    bg = f32(inp["bih_g"]) + f32(inp["bhh_g"])

    def pad256(wt):
        """Zero-pad [IN+1, 1024] to [256, 1024] so the second contraction
        tile is a full 128 rows (FWL-eligible LDWEIGHTS; the zero rows make
        any garbage in the padded moving-operand rows contribute 0)."""
        p = np.zeros((256, 1024), np.float32)
        p[:wt.shape[0]] = wt
        return p

    # constraint input weights: 129 features + bias row, padded to 256
    out["wihc"] = pad256(gates(inp["Wih_c"], bc)).astype(bfloat16)
    # constraint recurrent [256, 1024], consumes H2 -> extra 0.5
    whhc = (f32(inp["Whh_c"])[GATE_PERM] * ROW_SCALE[:, None]).T * 0.5
    out["whhc"] = np.ascontiguousarray(whhc).astype(bfloat16)

    wg = f32(inp["Wih_g"])[GATE_PERM] * ROW_SCALE[:, None]   # [1024, 384]
    # gen x-part: 128 features + bias row, padded to 256
    out["wgx"] = pad256(np.concatenate(
        [wg[:, :F].T, (bg[GATE_PERM] * ROW_SCALE)[None, :]], axis=0
    )).astype(bfloat16)
    # gen hc-part [256, 1024], consumes H2c -> extra 0.5
    out["wghc"] = np.ascontiguousarray(wg[:, F:].T * 0.5).astype(bfloat16)
    whhg = (f32(inp["Whh_g"])[GATE_PERM] * ROW_SCALE[:, None]).T * 0.5
    out["whhg"] = np.ascontiguousarray(whhg).astype(bfloat16)

    # MLP head; W1 consumes H2g -> 0.5
    out["w1t"] = np.ascontiguousarray(f32(inp["W1"]).T * 0.5).astype(bfloat16)
    out["w2t"] = np.ascontiguousarray(f32(inp["W2"]).T).astype(bfloat16)
    out["b1"] = np.ascontiguousarray(f32(inp["b1"])[:, None])
    out["b2"] = np.ascontiguousarray(f32(inp["b2"])[:, None])
    return out


def prep_core_inputs(seq, seq_constraints, c0, c1, s):
    """Per-core activation tensors for batch columns [c0:c1), seq len s."""
    bl = c1 - c0
    xc = np.asarray(seq_constraints, np.float32)[:s, c0:c1]   # [s, bl, 129]
    # time-reversed + transposed: xcT[k, tau, b] = xc[s-1-tau, b, k]
    xcT = np.empty((130, s, bl), np.float32)
    xcT[:129] = xc[::-1].transpose(2, 0, 1)
    xcT[129] = 1.0                                            # bias ones-row
    sq = np.asarray(seq, np.float32)[:s, c0:c1]               # [s, bl, 128]
    xgT = np.empty((129, s, bl), np.float32)
    xgT[0:128, 0] = 0.0
    xgT[0:128, 1:] = sq[:-1].transpose(2, 0, 1)
    xgT[128] = 1.0
    return {"xcT": xcT.astype(bfloat16), "xgT": xgT.astype(bfloat16)}


# --------------------------------------------------------------------------
# device program
# --------------------------------------------------------------------------

def build_program(s=S_FULL, tseg=TSEG, bl=BL):
    """Build + compile the per-core Bass program. Returns (nc, out_name)."""
    assert s % tseg == 0 and s % TMLP == 0
    nseg = s // tseg
    nc = bacc.Bacc("TRN2", target_bir_lowering=False, debug=False,
                   enable_asserts=False)

    d_xcT = nc.dram_tensor("xcT", [130, s, bl], BF16, kind="ExternalInput")
    d_xgT = nc.dram_tensor("xgT", [129, s, bl], BF16, kind="ExternalInput")
    d_wihc = nc.dram_tensor("wihc", [256, 4 * H], BF16, kind="ExternalInput")
    d_whhc = nc.dram_tensor("whhc", [H, 4 * H], BF16, kind="ExternalInput")
    d_wgx = nc.dram_tensor("wgx", [256, 4 * H], BF16, kind="ExternalInput")
    d_wghc = nc.dram_tensor("wghc", [H, 4 * H], BF16, kind="ExternalInput")
    d_whhg = nc.dram_tensor("whhg", [H, 4 * H], BF16, kind="ExternalInput")
    d_w1t = nc.dram_tensor("w1t", [H, F], BF16, kind="ExternalInput")
    d_w2t = nc.dram_tensor("w2t", [F, F], BF16, kind="ExternalInput")
    d_b1 = nc.dram_tensor("b1", [128, 1], F32, kind="ExternalInput")
    d_b2 = nc.dram_tensor("b2", [128, 1], F32, kind="ExternalInput")
    d_out = nc.dram_tensor("out", [F, s, bl], F32, kind="ExternalOutput")

    with tile.TileContext(nc) as tc, ExitStack() as ctx:
        wp = ctx.enter_context(tc.tile_pool(name="weights", bufs=1))
        hcp = ctx.enter_context(tc.tile_pool(name="hstore", bufs=1))
        xinp = ctx.enter_context(tc.tile_pool(name="xin", bufs=3))
        stp = ctx.enter_context(tc.tile_pool(name="state", bufs=4))
        vup = ctx.enter_context(tc.tile_pool(name="vu", bufs=3))
        tcp = ctx.enter_context(tc.tile_pool(name="tcell", bufs=3))
        yp = ctx.enter_context(tc.tile_pool(name="yout", bufs=3))

        # ---- load weights to SBUF (resident all kernel) ----
        def wtile(dram, shape, dt=BF16, row0=0, tag=None):
            t = wp.tile(shape, dt, tag=tag or f"w_{dram.name}_{row0}",
                         name=f"w_{dram.name}_{row0}")
            nc.sync.dma_start(t[:], dram.ap()[row0:row0 + shape[0]])
            return t

        wihc0 = wtile(d_wihc, [128, 4 * H])
        wihc1 = wtile(d_wihc, [128, 4 * H], row0=128)
        whhc = [wtile(d_whhc, [128, 4 * H], row0=128 * k) for k in range(2)]
        wgx0 = wtile(d_wgx, [128, 4 * H])
        wgx1 = wtile(d_wgx, [128, 4 * H], row0=128)
        wghc = [wtile(d_wghc, [128, 4 * H], row0=128 * k) for k in range(2)]
        whhg = [wtile(d_whhg, [128, 4 * H], row0=128 * k) for k in range(2)]
        w1t = [wtile(d_w1t, [128, F], row0=128 * k) for k in range(2)]
        w2t = wtile(d_w2t, [128, F])
        b1_sb = wtile(d_b1, [128, 1], F32)
        b2_sb = wtile(d_b2, [128, 1], F32)

        # hidden stores (H2 = 2*h, bf16), per hidden half
        hc = [hcp.tile([128, s, bl], BF16, tag=f"hc{k}", name=f"hc{k}")
              for k in range(2)]
        hg = [hcp.tile([128, s, bl], BF16, tag=f"hg{k}", name=f"hg{k}")
              for k in range(2)]

        # zero h for step 0
        hz = hcp.tile([128, bl], BF16, tag="hz", name="hz")
        nc.vector.memset(hz[:], 0.0)

        # padded second-contraction-tile inputs (rows >= x1_rows stay 0 from
        # the one-time memset; the matching weight rows are 0 anyway, the
        # zeroing just guards against NaN garbage)
        x1t = [hcp.tile([128, tseg, bl], BF16, tag=f"x1_{i}", name=f"x1_{i}")
               for i in range(3)]
        for t_ in x1t:
            nc.vector.memset(t_[:], 0.0)

        def scan_phase(psb, d_x, x1_rows, wih0, wih1, whh, hstore,
                       reverse, hc_bulk):
            """One LSTM scan over the full sequence."""

            def seg_dma(seg):
                """Issue input DMAs for a segment (2 segments ahead of use,
                so bulk matmuls never stall the in-order tensor queue)."""
                t0 = seg * tseg
                x0 = xinp.tile([128, tseg, bl], BF16, tag="x0", name="x0")
                nc.sync.dma_start(x0[:], d_x.ap()[0:128, t0:t0 + tseg])
                x1 = x1t[seg % 3]
                nc.sync.dma_start(x1[0:x1_rows],
                                  d_x.ap()[128:128 + x1_rows, t0:t0 + tseg])
                return x0, x1

            def seg_mms(seg, x0, x1):
                """Allocate gate PSUM for a segment; return the (not yet
                emitted) bulk matmul argument list."""
                t0 = seg * tseg
                psA = [psb.tile([128, 4, tseg, bl], F32, tag=f"psA{h}",
                                name=f"psA{h}") for h in range(2)]
                mms = []
                for h in range(2):
                    for qq in range(4):
                        c = 128 * (4 * h + qq)
                        # start=True on the first write to each PSUM bank
                        mms.append((psA[h][:, qq], wih0[:, c:c + 128], x0[:],
                                    qq % 2 == 0))
                        mms.append((psA[h][:, qq], wih1[:, c:c + 128], x1[:],
                                    False))
                        if hc_bulk is not None:
                            for k in range(2):
                                mms.append((psA[h][:, qq],
                                            hc_bulk[1][k][:, c:c + 128],
                                            hc_bulk[0][k][:, t0:t0 + tseg],
                                            False))
                return psA, mms

            def emit_bulk(mms):
                for out, lhsT, rhs, start in mms:
                    nc.tensor.matmul(out, lhsT, rhs, start=start, stop=False,
                                     skip_group_check=True)

            h_prev = [hz[:], hz[:]]
            st_cur = []
            for h in range(2):
                st = stp.tile([128, 5, bl], F32, tag=f"st{h}", name=f"st{h}")
                nc.vector.memset(st[:, 0, :], 0.0)      # cs_0 = 0
                st_cur.append(st)

            xt = {0: seg_dma(0)}
            if nseg > 1:
                xt[1] = seg_dma(1)
            psA, mms = seg_mms(0, *xt[0])
            emit_bulk(mms)
            for seg in range(nseg):
                if seg + 2 < nseg:
                    xt[seg + 2] = seg_dma(seg + 2)
                if seg + 1 < nseg:
                    psA_n, mms_n = seg_mms(seg + 1, *xt.pop(seg + 1))
                else:
                    psA_n, mms_n = None, []
                # next segment's bulk matmuls, interleaved into this
                # segment's steps to fill tensor-engine idle time
                chunk = -(-len(mms_n) // tseg) if mms_n else 0

                for tl in range(tseg):
                    t = seg * tseg + tl
                    t_out = (s - 1 - t) if reverse else t
                    hp = list(h_prev)
                    st_next = [stp.tile([128, 5, bl], F32, tag=f"st{h}",
                                        name=f"stn{h}")
                               for h in range(2)]
                    for h in range(2):
                        for k in range(2):
                            for qq in range(4):
                                c = 128 * (4 * h + qq)
                                nc.tensor.matmul(
                                    psA[h][:, qq, tl], whh[k][:, c:c + 128],
                                    hp[k], start=False, stop=(k == 1),
                                    skip_group_check=True)
                    emit_bulk(mms_n[tl * chunk:(tl + 1) * chunk])
                    for h in range(2):
                        st = st_cur[h]
                        # [tg, tf, ti, to] <- tanh(gate pre-acts)
                        nc.scalar.activation(st[:, 1:5], psA[h][:, :, tl],
                                             AF.Tanh)
                        vu = vup.tile([128, 2, bl], F32, tag=f"vu{h}",
                                      name=f"vu{h}")
                        # v2 = (tf+1)*cs ; u2 = (ti+1)*tg
                        nc.vector.scalar_tensor_tensor(
                            vu[:], st[:, 2:4], 1.0, st[:, 0:2],
                            ALU.add, ALU.mult)
                        # cs_new = v2*0.5 + u2
                        nc.vector.scalar_tensor_tensor(
                            st_next[h][:, 0], vu[:, 0], 0.5, vu[:, 1],
                            ALU.mult, ALU.add)
                        tcl = tcp.tile([128, bl], F32, tag=f"tc{h}",
                                       name=f"tc{h}")
                        nc.scalar.activation(tcl[:], st_next[h][:, 0],
                                             AF.Tanh, scale=0.5)
                        # H2 = (to+1)*tc -> bf16 hidden store
                        nc.vector.scalar_tensor_tensor(
                            hstore[h][:, t_out], st[:, 4], 1.0, tcl[:],
                            ALU.add, ALU.mult)
                        h_prev[h] = hstore[h][:, t_out]
                    st_cur = st_next
                psA = psA_n

        with tc.tile_pool(name="psscan", bufs=2, space="PSUM") as psb:
            # phase C: constraint LSTM, backward in time
            scan_phase(psb, d_xcT, 2, wihc0, wihc1, whhc, hc,
                       reverse=True, hc_bulk=None)
            # phase G: generation LSTM, forward
            scan_phase(psb, d_xgT, 1, wgx0, wgx1, whhg, hg,
                       reverse=False, hc_bulk=(hc, wghc))

        # ---- phase M: bulk MLP head over all stored hg ----
        with tc.tile_pool(name="psmlp", bufs=4, space="PSUM") as psm:
            for t0 in range(0, s, TMLP):
                ps1 = psm.tile([128, TMLP, bl], F32, tag="ps1", name="ps1")
                for k in range(2):
                    nc.tensor.matmul(ps1[:], w1t[k][:],
                                     hg[k][:, t0:t0 + TMLP],
                                     start=(k == 0), stop=(k == 1))
                y1 = yp.tile([128, TMLP, bl], BF16, tag="y1", name="y1")
                nc.scalar.activation(y1[:], ps1[:], AF.Relu,
                                     bias=b1_sb[:, 0:1])
                ps2 = psm.tile([128, TMLP, bl], F32, tag="ps2", name="ps2")
                nc.tensor.matmul(ps2[:], w2t[:], y1[:], start=True, stop=True)
                y2 = yp.tile([128, TMLP, bl], F32, tag="y2", name="y2")
                nc.scalar.activation(y2[:], ps2[:], AF.Identity,
                                     bias=b2_sb[:, 0:1])
                nc.sync.dma_start(d_out.ap()[:, t0:t0 + TMLP], y2[:])

    nc.compile()
    return nc, "out"


_PROGRAM_CACHE = {}


def get_program(s=S_FULL, tseg=TSEG, bl=BL):
    key = (s, tseg, bl)
    if key not in _PROGRAM_CACHE:
        _PROGRAM_CACHE[key] = build_program(s, tseg, bl)
    return _PROGRAM_CACHE[key]


# --------------------------------------------------------------------------
# entry point
# --------------------------------------------------------------------------

def kernel(**inputs) -> np.ndarray:
    s, b = np.asarray(inputs["seq"]).shape[:2]
    assert (s, b) == (S_FULL, B_FULL)
    nc, out_name = get_program()
    w = prep_weights(inputs)
    in_maps = []
    for core in range(NCORES):
        c0 = core * BL
        m = dict(w)
        m.update(prep_core_inputs(inputs["seq"], inputs["seq_constraints"],
                                  c0, c0 + BL, S_FULL))
        in_maps.append(m)
    res = run_bass_kernel_spmd(nc, in_maps, core_ids=list(range(NCORES)))
    # per-core out: [F, S, BL] -> [S, BL, F]; concat cores along batch
    parts = [np.transpose(res.results[c][out_name], (1, 2, 0))
             for c in range(NCORES)]
    return np.ascontiguousarray(np.concatenate(parts, axis=1))
